# revision 18
# baseline (speedup 1.0000x reference)
"""Trainium2 Bass kernel: gated causal self-attention (GQA + partial RoPE).

Reference computation (per batch):
    q,k,v = x@Wq, x@Wk, x@Wv  (heads split, partial RoPE on first R dims)
    att = softmax(causal(q k^T / sqrt(D)))
    att = att * (att >= sigmoid(gate))          # post-softmax threshold gate
    y = (att @ v) @ Wo

Sharding over 8 NeuronCores: core = 4*b + g where b in {0,1} is the batch
(data parallel) and g in {0..3} is the KV-head group (tensor parallel:
Wq/Wk/Wv column-sharded, Wo row-sharded; gate sharded with heads).  Each
core computes a partial y^T (C x T); the host sums the 4 group partials
per batch and transposes.  The TxT score tensor never leaves a core.

v3: the kernel is PE-stream-bound (~242us of matmul columns at full
clock), so the emission order keeps the PE saturated:
  - h-major scores per t-block with den trailing one head and att@v two
    heads behind, so the ACT exp stream (2.7x slower than the score
    stream) never stalls the PE;
  - "filler" matmuls -- the NEXT block's projections (group-major over a
    single resident x block) and PREVIOUS blocks' output projections --
    are pumped between score/den/av groups; output-projection fillers
    may spill across one block boundary (ytb double-buffered);
  - block-0 projections run chunk-major so the PE consumes c-tiles in
    DMA arrival order during the 10MB startup load;
  - gating compares es >= thr*den (both 2x-mode DVE passes), y
    normalized after att@v as in the baseline;
  - causal masks are iota-predicated affine_select ops on the otherwise
    idle GpSimd engine; output-projection PSUM drains go to DVE; q/k run
    in f16 (scores band ~7e-4, comparable to the f16 es rounding).

PSUM budget (8 banks): sp x3 (score tiles), dn x1 (denominator, shared
with att@v accumulator -- they are sequential per head), prj x2
(projection group accumulators), acc x2 (outproj tiles + v-transposes).
"""

from collections import deque

import numpy as np

import concourse.bass as bass
import concourse.tile as tile
from concourse import bacc, mybir
from concourse.alu_op_type import AluOpType
from concourse.bass_utils import run_bass_kernel_spmd

B, T, C = 2, 2048, 2048
H, HKV, D = 16, 4, 128
R = 64
NCORE = 8
G = 4            # tensor-parallel degree over KV heads
HL = H // G      # 4 local q heads per core
DL = HL * D      # 512 local q dims per core
SCALE = float(D) ** -0.5

F32 = mybir.dt.float32
F32R = mybir.dt.float32r
F16 = mybir.dt.float16
EXP = mybir.ActivationFunctionType.Exp

TB = 512                 # t-block width
NTB = T // TB            # 4
NCT = C // 128           # 16 contraction tiles
GB = 4                   # s-tiles per batched DVE gating op

EYE0, THR0 = 0, 128
CONST_W = 128 + HL


def build():
    nc = bacc.Bacc("TRN2", target_bir_lowering=False, debug=False)

    xT = nc.dram_tensor("xT", [C, T], F16, kind="ExternalInput").ap()
    wq = nc.dram_tensor("wq", [C, DL], F16, kind="ExternalInput").ap()
    wk = nc.dram_tensor("wk", [C, D], F16, kind="ExternalInput").ap()
    wv = nc.dram_tensor("wv", [C, D], F16, kind="ExternalInput").ap()
    wo = nc.dram_tensor("wo", [DL, C], F16, kind="ExternalInput").ap()
    ones = nc.dram_tensor("ones", [128, 128], F16, kind="ExternalInput").ap()
    cs = nc.dram_tensor("cs", [R, T], F16, kind="ExternalInput").ap()
    sn = nc.dram_tensor("sn", [R, T], F16, kind="ExternalInput").ap()
    cst = nc.dram_tensor("cst", [128, CONST_W], F32, kind="ExternalInput").ap()
    ypT = nc.dram_tensor("ypT", [C, T], F16, kind="ExternalOutput").ap()

    with tile.TileContext(nc) as tc:
        with (
            tc.tile_pool(name="persist", bufs=1) as persist,
            tc.tile_pool(name="wpool", bufs=1) as wpool,
            tc.tile_pool(name="xpool", bufs=2) as xpool,
            tc.tile_pool(name="espool", bufs=1) as espool,
            tc.tile_pool(name="qpool", bufs=2) as qpool,
            tc.tile_pool(name="ypool", bufs=2) as ypool,
            tc.tile_pool(name="small", bufs=1) as small,
            tc.tile_pool(name="psum", bufs=1, space="PSUM") as psum,
        ):
            # ---- persistent SBUF ----
            kt = persist.tile([128, T], F16)     # k^T (D x T), rope applied
            vn = persist.tile([128, T], F16)     # v natural; s-tile i at cols 128i
            cs_sb = persist.tile([R, T], F16)    # cos^T
            sn_sb = persist.tile([R, T], F16)    # [-sinT[0:32] ; sinT[32:64]]
            ones_sb = persist.tile([128, 128], F16)
            cst_sb = persist.tile([128, CONST_W], F32)
            eye_sb = cst_sb[:, EYE0 : EYE0 + 128]

            wq_sb = wpool.tile([128, NCT, DL], F16, tag="wq", name="wq_sb")
            wk_sb = wpool.tile([128, NCT, D], F16, tag="wk", name="wk_sb")
            wv_sb = wpool.tile([128, NCT, D], F16, tag="wv", name="wv_sb")
            wo_sb = wpool.tile([128, HL, C], F16, tag="wo", name="wo_sb")
            xss = {0: xpool.tile([128, NCT, TB], F16, tag="xs", name="xs_0")}
            xs = xss[0]

            # startup DMAs in PE consumption order (SP issues ~1.6/us, so
            # pace [xs[c], wq[c]] pairs with proj0's per-c consumption);
            # wk/wv batched per 4 c-tiles, tables mid-stream, xs_1 + wo last
            wkr = wk.rearrange("(i p) f -> p i f", p=128)
            wvr = wv.rearrange("(i p) f -> p i f", p=128)
            for c in range(NCT):
                csl = slice(128 * c, 128 * (c + 1))
                nsp = 4 if c == 0 else 1
                for u in range(nsp):
                    fsl = slice(TB * u // nsp, TB * (u + 1) // nsp)
                    nc.sync.dma_start(xs[:, c, fsl], xT[csl, fsl])
                    nc.sync.dma_start(wq_sb[:, c, fsl], wq[csl, fsl])
                if c % 4 == 3:
                    i4 = slice(c - 3, c + 1)
                    nc.sync.dma_start(wk_sb[:, i4, :], wkr[:, i4, :])
                    nc.sync.dma_start(wv_sb[:, i4, :], wvr[:, i4, :])
                if c == 7:
                    nc.sync.dma_start(cs_sb[:], cs)
                    nc.sync.dma_start(sn_sb[:], sn)
                    nc.sync.dma_start(ones_sb[:], ones)
                    nc.sync.dma_start(cst_sb[:], cst)
            xs1 = xpool.tile([128, NCT, TB], F16, tag="xs", name="xs_1")
            xss[1] = xs1
            for c in range(NCT):
                nc.sync.dma_start(xs1[:, c, :], xT[128 * c : 128 * (c + 1), TB : 2 * TB])
            for d in range(HL):
                for u in range(2):
                    fsl = slice(C * u // 2, C * (u + 1) // 2)
                    nc.sync.dma_start(wo_sb[:, d, fsl], wo[128 * d : 128 * (d + 1), fsl])

            es = [
                espool.tile([128, NTB * 4 * TB], F16, tag=f"es{h}", name=f"es{h}")
                for h in range(HL)
            ]
            qtbs = {}
            ytbs = {}

            # ---- PE filler machinery ----
            fillers = deque()  # (kind, idx, closure); closure emits ~4 matmuls

            def pump(n=1):
                for _ in range(n):
                    if fillers:
                        fillers.popleft()[2]()

            def flush_outs_upto(oj_max):
                rest = []
                for it in fillers:
                    if it[0] == "out" and it[1] <= oj_max:
                        it[2]()
                    else:
                        rest.append(it)
                fillers.clear()
                fillers.extend(rest)

            def flush_projs():
                rest = [it for it in fillers if it[0] != "proj"]
                for it in fillers:
                    if it[0] == "proj":
                        it[2]()
                fillers.clear()
                fillers.extend(rest)

            def rope(th, dcols, tcols, name):
                """partial RoPE in place on rows 0:R of th[:, dcols] (f16)."""
                hw = R // 2
                rot = small.tile([R, TB], F16, tag="rot", bufs=1, name=f"rot_{name}")
                nc.scalar.copy(rot[0:hw, :], th[hw:R, dcols])
                nc.scalar.copy(rot[hw:R, :], th[0:hw, dcols])
                nc.vector.tensor_tensor(
                    th[0:R, dcols], th[0:R, dcols], cs_sb[:, tcols], op=AluOpType.mult
                )
                nc.vector.tensor_tensor(
                    rot[:], rot[:], sn_sb[:, tcols], op=AluOpType.mult
                )
                nc.vector.tensor_tensor(
                    th[0:R, dcols], th[0:R, dcols], rot[:], op=AluOpType.add
                )

            def new_qtb(pj):
                qtbs[pj] = qpool.tile([128, HL * TB], F16, tag="qtb", name=f"qtb_{pj}")
                return qtbs[pj]

            def drain_q(gp, qtb, h, tsl, pj):
                dsl = slice(TB * h, TB * (h + 1))
                nc.scalar.copy(qtb[:, dsl], gp[:])
                rope(qtb, dsl, tsl, f"q{pj}_{h}")

            def drain_k(gp, tsl, pj):
                nc.scalar.copy(kt[:, tsl], gp[:])
                rope(kt, tsl, tsl, f"k{pj}")

            def vtrans(vt, pj):
                tp = psum.tile([128, TB], F32, tag="acc", bufs=2, name=f"tp_{pj}")
                for u in range(TB // 128):
                    nc.tensor.transpose(
                        tp[:, 128 * u : 128 * (u + 1)], vt[:, 128 * u : 128 * (u + 1)], eye_sb
                    )
                s0 = pj * 4
                nc.vector.tensor_copy(vn[:, 128 * s0 : 128 * (s0 + 4)], tp[:])

            def make_proj_units(pj):
                """Filler units computing q/k/v for block pj from xss[pj]
                (group-major: <=2 'prj' PSUM accumulators live)."""
                tsl = slice(pj * TB, (pj + 1) * TB)
                xs = xss[pj]
                qtb = new_qtb(pj)
                units = []

                def group(w_sb, col0, ncols, drain):
                    gp = psum.tile(
                        [128, TB], F32, tag="prj", bufs=2, name=f"prj_{pj}_{col0}_{ncols}"
                    )
                    for cu in range(4):
                        def u(gp=gp, cu=cu, w_sb=w_sb, col0=col0, ncols=ncols, drain=drain):
                            for c in range(4 * cu, 4 * cu + 4):
                                nc.tensor.matmul(
                                    gp[:],
                                    w_sb[:, c, col0 : col0 + ncols],
                                    xs[:, c, :],
                                    start=(c == 0),
                                    stop=(c == NCT - 1),
                                )
                            if cu == 3:
                                drain(gp)
                        units.append(("proj", pj, u))

                group(wk_sb, 0, D, lambda gp: drain_k(gp, tsl, pj))
                for h in range(HL):
                    group(wq_sb, 128 * h, 128,
                          lambda gp, h=h: drain_q(gp, qtb, h, tsl, pj))
                vt = small.tile([128, TB], F32, tag="vt", bufs=1, name=f"vt_{pj}")

                def vdrain(gp):
                    nc.scalar.copy(vt[:], gp[:])
                group(wv_sb, 0, D, vdrain)
                units.append(("proj", pj, lambda: vtrans(vt, pj)))
                return units

            def emit_proj0():
                """Block-0 projections, chunk-major (matches DMA arrival).
                All PSUM banks are free at startup: qp0-2 on 'sp', qp3 on
                'dn', kp/vp on 'prj'."""
                tsl = slice(0, TB)
                qtb = new_qtb(0)
                qps = [
                    psum.tile([128, TB], F32, tag=("sp" if h < 3 else "dn"),
                              bufs=(3 if h < 3 else 1), name=f"qp0_{h}")
                    for h in range(HL)
                ]
                kp = psum.tile([128, TB], F32, tag="prj", bufs=2, name="kp0")
                vp = psum.tile([128, TB], F32, tag="prj", bufs=2, name="vp0")
                groups = [(qps[h], wq_sb, 128 * h, 128) for h in range(HL)]
                groups += [(kp, wk_sb, 0, D), (vp, wv_sb, 0, D)]
                for ch in range(4):
                    for gp, w_sb, col0, ncols in groups:
                        for ci in range(4):
                            c = 4 * ch + ci
                            nc.tensor.matmul(
                                gp[:], w_sb[:, c, col0 : col0 + ncols], xs[:, c, :],
                                start=(c == 0), stop=(c == NCT - 1),
                            )
                for h in range(HL):
                    drain_q(qps[h], qtb, h, tsl, 0)
                drain_k(kp, tsl, 0)
                vt = small.tile([128, TB], F32, tag="vt", bufs=1, name="vt_0")
                nc.scalar.copy(vt[:], vp[:])
                vtrans(vt, 0)

            def make_outproj_units(oj):
                """Output projection for block oj; reads ytbs[oj] (ytb is
                double-buffered, so these may run one block late)."""
                tsl = slice(oj * TB, (oj + 1) * TB)
                ytb = ytbs[oj]
                units = []
                for co in range(NCT):
                    def u(co=co):
                        op = psum.tile([128, TB], F32, tag="acc", bufs=2, name=f"op_{oj}_{co}")
                        for d in range(HL):
                            nc.tensor.matmul(
                                op[:],
                                wo_sb[:, d, 128 * co : 128 * (co + 1)],
                                ytb[:, TB * d : TB * (d + 1)],
                                start=(d == 0),
                                stop=(d == HL - 1),
                            )
                        stg = small.tile([128, TB], F16, tag="stg", bufs=2, name=f"stg_{oj}_{co}")
                        # alternate the PSUM drain between ACT and DVE so
                        # neither engine's queue backs up behind exp/gating
                        if co % 2 == 0:
                            nc.scalar.copy(stg[:], op[:])
                        else:
                            nc.vector.tensor_copy(stg[:], op[:])
                        nsp = 2 if oj == NTB - 1 else 1
                        for u in range(nsp):
                            fsl = slice(TB * u // nsp, TB * (u + 1) // nsp)
                            nc.sync.dma_start(
                                ypT[128 * co : 128 * (co + 1), tsl][:, fsl], stg[:, fsl]
                            )
                    units.append(("out", oj, u))
                return units

            emit_proj0()

            rdens = {}

            def emit_scores(j, h):
                qtb = qtbs[j]
                nst = 4 * j + 4
                qsl = slice(TB * h, TB * (h + 1))
                for i in range(nst):
                    sp = psum.tile([128, TB], F32, tag="sp", bufs=3, name=f"sp_{j}_{h}_{i}")
                    nc.tensor.matmul(
                        sp[:], kt[:, 128 * i : 128 * (i + 1)], qtb[:, qsl],
                        start=True, stop=True,
                    )
                    est = es[h][:, TB * i : TB * (i + 1)]
                    nc.scalar.activation(est, sp[:], EXP, scale=SCALE)
                    dpos = i - 4 * j
                    if dpos >= 0:
                        # causal: keep where t_glob >= s_glob, i.e.
                        # (512j - 128i) - p + f >= 0
                        nc.gpsimd.affine_select(
                            out=est,
                            in_=est,
                            pattern=[[1, TB]],
                            compare_op=AluOpType.is_ge,
                            fill=0.0,
                            base=512 * j - 128 * i,
                            channel_multiplier=-1,
                        )
                    if i % 2 == 1:
                        pump(1)

            def emit_den(j, h):
                nst = 4 * j + 4
                pump(2)
                dn = psum.tile([128, TB], F32, tag="dn", bufs=1, name=f"dn_{j}_{h}")
                for i in range(nst):
                    nc.tensor.matmul(
                        dn[:], ones_sb, es[h][:, TB * i : TB * (i + 1)],
                        start=(i == 0), stop=(i == nst - 1),
                    )
                    if i % 4 == 3:
                        pump(1)
                cthr = small.tile([128, TB], F16, tag="cthr", bufs=1, name=f"ct_{j}_{h}")
                rden = small.tile([128, TB], F32, tag="rden", bufs=3, name=f"rd_{j}_{h}")
                rdens[(j, h)] = rden
                nc.vector.tensor_scalar_mul(
                    cthr[:], dn[:], cst_sb[:, THR0 + h : THR0 + h + 1]
                )
                nc.vector.reciprocal_approx_fast(out=rden[:], in_=dn[:])
                for g0 in range(0, nst, GB):
                    gn = min(GB, nst - g0)
                    ev = es[h][:, TB * g0 : TB * (g0 + gn)].rearrange(
                        "p (r n) -> p r n", r=gn
                    )
                    cb = cthr[:][:, None, :].broadcast_to([128, gn, TB])
                    msk = small.tile(
                        [128, GB * TB], F16, tag="msk", bufs=2, name=f"mk_{j}_{h}_{g0}"
                    )
                    mv = msk[:, 0 : TB * gn].rearrange("p (r n) -> p r n", r=gn)
                    nc.vector.tensor_tensor(mv, ev, cb, op=AluOpType.is_ge)
                    nc.vector.tensor_tensor(ev, ev, mv, op=AluOpType.mult)

            def emit_av(j, h):
                nst = 4 * j + 4
                pump(4)
                yp = psum.tile([128, TB], F32, tag="dn", bufs=1, name=f"yp_{j}_{h}")
                for i in range(nst):
                    nc.tensor.matmul(
                        yp[:], vn[:, 128 * i : 128 * (i + 1)],
                        es[h][:, TB * i : TB * (i + 1)],
                        start=(i == 0), stop=(i == nst - 1),
                    )
                    if i % 4 == 3:
                        pump(1)
                nc.vector.tensor_tensor(
                    ytbs[j][:, TB * h : TB * (h + 1)], yp[:], rdens[(j, h)][:],
                    op=AluOpType.mult,
                )

            # flat task stream: scores at t, den trails 1 task, av trails 2;
            # block boundaries only gate projections/buffers, not the
            # exp/gating pipelines
            NT = 4 * NTB

            def on_task(t):
                j, h = t // 4, t % 4
                if h == 0:
                    # proj_j must be complete before scores (j,0)
                    flush_projs()
                    ytbs[j] = ypool.tile([128, HL * TB], F16, tag="ytb", name=f"ytb_{j}")
                    if j + 2 < NTB:
                        xs_n = xpool.tile([128, NCT, TB], F16, tag="xs", name=f"xs_{j+2}")
                        xss[j + 2] = xs_n
                        tc0 = (j + 2) * TB
                        for c in range(NCT):
                            nc.sync.dma_start(
                                xs_n[:, c, :],
                                xT[128 * c : 128 * (c + 1), tc0 : tc0 + TB],
                            )
                    if j + 1 < NTB:
                        fillers.extend(make_proj_units(j + 1))
                emit_scores(j, h)

            for t in range(NT):
                on_task(t)
                if t >= 1:
                    emit_den((t - 1) // 4, (t - 1) % 4)
                if t >= 2:
                    ta = t - 2
                    ja, ha = ta // 4, ta % 4
                    if ha == 0:
                        # av writes ytb buffer ja%2: everything still reading
                        # ytb_{ja-2} (same buffer) must be emitted first
                        flush_outs_upto(ja - 2)
                    emit_av(ja, ha)
                    if ha == HL - 1:
                        fillers.extend(make_outproj_units(ja))

            emit_den(NTB - 1, HL - 1)
            emit_av(NTB - 1, HL - 2)
            emit_av(NTB - 1, HL - 1)
            fillers.extend(make_outproj_units(NTB - 1))
            while fillers:
                fillers.popleft()[2]()

    nc.compile()
    return nc


_NC_CACHE = None


def _get_nc():
    global _NC_CACHE
    if _NC_CACHE is None:
        _NC_CACHE = build()
    return _NC_CACHE


def make_in_maps(x, cos, sin, Wq, Wk, Wv, Wo, gate):
    x = np.asarray(x, np.float32)
    cos = np.asarray(cos, np.float32)
    sin = np.asarray(sin, np.float32)
    Wq = np.asarray(Wq, np.float32)
    Wk = np.asarray(Wk, np.float32)
    Wv = np.asarray(Wv, np.float32)
    Wo = np.asarray(Wo, np.float32)
    gate = np.asarray(gate, np.float32)

    hw = R // 2
    cosT = np.ascontiguousarray(cos.T).astype(np.float16)  # (R, T)
    sinT = sin.T
    sn_signed = np.ascontiguousarray(
        np.concatenate([-sinT[0:hw], sinT[hw:R]], axis=0)
    ).astype(np.float16)
    thr_full = 1.0 / (1.0 + np.exp(-gate))  # sigmoid, (H,)
    cst_base = np.zeros((128, CONST_W), np.float32)
    cst_base[:, EYE0 : EYE0 + 128] = np.eye(128, dtype=np.float32)
    ones16 = np.ones((128, 128), np.float16)

    in_maps = []
    for core in range(NCORE):
        b, g = divmod(core, G)
        cst = cst_base.copy()
        cst[:, THR0 : THR0 + HL] = thr_full[HL * g : HL * (g + 1)]
        in_maps.append(
            {
                "xT": np.ascontiguousarray(x[b].T).astype(np.float16),
                "wq": np.ascontiguousarray(Wq[:, DL * g : DL * (g + 1)]).astype(np.float16),
                "wk": np.ascontiguousarray(Wk[:, D * g : D * (g + 1)]).astype(np.float16),
                "wv": np.ascontiguousarray(Wv[:, D * g : D * (g + 1)]).astype(np.float16),
                "wo": np.ascontiguousarray(Wo[DL * g : DL * (g + 1), :].astype(np.float16)),
                "ones": ones16,
                "cs": cosT,
                "sn": sn_signed,
                "cst": cst,
            }
        )
    return in_maps


def run(inputs, trace=False, **kw):
    """Run on 8 NeuronCores; returns (y_full, BassKernelResults)."""
    nc = _get_nc()
    in_maps = make_in_maps(**inputs)
    res = run_bass_kernel_spmd(nc, in_maps, core_ids=list(range(NCORE)), trace=trace, **kw)
    y = np.zeros((B, T, C), np.float32)
    for core in range(NCORE):
        b = core // G
        y[b] += res.results[core]["ypT"].T.astype(np.float32)
    return y, res


def kernel(**inputs) -> np.ndarray:
    y, _ = run(inputs)
    return y
